# revision 12
# baseline (speedup 1.0000x reference)
"""GRU (hidden_size=1) kernel for Trainium2, data-parallel over batch on 8 cores.

Problem: x[2048, 128, 512] f32, gi = x @ w_ih.T + b_ih, then a sequential
GRU scan over T=128 with scalar hidden state per batch element, output is
mean over batch of h_t -> [128].

Strategy per core (B_loc = 256 batch elements):
  - host pre-arranges the core's x slice as xt[T, D, B_loc] so DMA delivers
    [d, b] tiles directly (contraction dim on partitions, no on-chip
    transpose of x).
  - w-stationary float32r matmuls (full-rate at N=256) produce
    giT[3, b] per t in PSUM, accumulated over 4 d-chunks.
  - tiny PE transposes flip giT[3, 128] -> [128, 3] per batch half, giving
    gi with batch on partitions.
  - fp32 GRU scan over t with batch on partitions ([128, 2] element ops).
  - partition-sum via ones-matmul gives per-(t, half) batch sums; host sums
    the 8 per-core partials and divides by B.
"""

import numpy as np

import concourse.bass as bass
import concourse.mybir as mybir
from concourse.bass_types import AP
from concourse.tile import TileContext
from concourse.bass_utils import run_bass_kernel_spmd

F32 = mybir.dt.float32
F32R = mybir.dt.float32r
AF = mybir.ActivationFunctionType
ALU = mybir.AluOpType

N_CORES = 8
B, T, D = 2048, 128, 512
B_LOC = B // N_CORES          # 256
NH = B_LOC // 128             # 2 batch halves per core
NCH = D // 128                # 4 contraction chunks
GI_CHUNK = 32                 # t-steps per PSUM->SBUF gi flush

_CACHE = {}


def _bcast4(h_sl):
    """[128, 2] AP -> [128, 2(bcast), 2] AP reading (h0,h1,h0,h1)."""
    return AP(
        tensor=h_sl.tensor,
        offset=h_sl.offset,
        ap=[h_sl.ap[0], [0, 2], h_sl.ap[1]],
    )


def build_nc():
    nc = bass.Bass(trn_type="TRN2")

    xt = nc.dram_tensor("xt", [T, D, B_LOC], F32, kind="ExternalInput")
    wT = nc.dram_tensor("wT", [D, 3], F32, kind="ExternalInput")
    cst = nc.dram_tensor("cst", [128, 6 + NH], F32, kind="ExternalInput")
    g3 = nc.dram_tensor("g3", [3, 4], F32, kind="ExternalInput")
    out = nc.dram_tensor("out", [1, T * NH], F32, kind="ExternalOutput")

    with TileContext(nc) as tc:
        with (
            tc.tile_pool(name="xpool", bufs=8) as xpool,
            tc.tile_pool(name="consts", bufs=1) as consts,
            tc.tile_pool(name="gits", bufs=4) as gits,
            tc.tile_pool(name="scan", bufs=1) as scan,
            tc.tile_pool(name="gtp", bufs=2, space="PSUM") as gtp,
            tc.tile_pool(name="gbp", bufs=2, space="PSUM") as gbp,
            tc.tile_pool(name="sump", bufs=1, space="PSUM") as sump,
        ):
            # ---- constants ----
            wT_sb = consts.tile([128, NCH, 3], F32R)
            nc.sync.dma_start(
                out=wT_sb,
                in_=wT[:].rearrange("(c p) g -> p c g", p=128).bitcast(F32R),
            )
            cst_sb = consts.tile([128, 6 + NH], F32)
            nc.sync.dma_start(out=cst_sb, in_=cst[:])
            g3_sb = consts.tile([3, 4], F32)
            nc.sync.dma_start(out=g3_sb, in_=g3[:])
            gb_sb = g3_sb[:, 0:1]
            id3_sb = g3_sb[:, 1:4]
            h0_sb = cst_sb[:, 6:6 + NH]
            ones_sb = consts.tile([128, 1], F32)
            nc.vector.memset(ones_sb, 1.0)

            # warm-up consumers: absorb each const-DMA semaphore into the
            # engines' vector clocks so later instructions need only one wait
            # (self-loading f32r matmuls have a single sync-wait slot).
            warm_ps = sump.tile([3, 3], F32, tag="warm")
            nc.tensor.ldweights(wT_sb[:, 0, :].bitcast(mybir.dt.bfloat16))
            nc.tensor.transpose(warm_ps, id3_sb, id3_sb)
            warm_sb = consts.tile([3, 1], F32)
            nc.scalar.copy(warm_sb, g3_sb[:, 0:1])

            # ---- persistent buffers ----
            gi_sb = scan.tile([128, T * 3 * NH], F32)   # col 6t+2g+half
            hout = scan.tile([128, T * NH], F32)        # col 2t+half

            # ---- gi production ----
            for tc_i in range(T // GI_CHUNK):
                gib = [
                    gbp.tile([128, GI_CHUNK * 3], F32, tag=f"gib{h}", name=f"gib{h}")
                    for h in range(NH)
                ]
                for tt in range(GI_CHUNK):
                    t = tc_i * GI_CHUNK + tt
                    x_sb = xpool.tile([128, NCH, B_LOC], F32R)
                    nc.sync.dma_start(
                        out=x_sb,
                        in_=xt[t].rearrange("(c p) b -> p c b", p=128).bitcast(F32R),
                    )
                    git_ps = gtp.tile([3, B_LOC], F32)
                    for c in range(NCH):
                        nc.tensor.matmul(
                            git_ps,
                            wT_sb[:, c, :],
                            x_sb[:, c, :],
                            start=(c == 0),
                            stop=(c == NCH - 1),
                        )
                    git_sb = gits.tile([3, B_LOC], F32)
                    nc.scalar.activation(
                        out=git_sb, in_=git_ps, func=AF.Identity,
                        bias=gb_sb, scale=1.0,
                    )
                    for h in range(NH):
                        nc.tensor.transpose(
                            gib[h][:, 3 * tt:3 * tt + 3],
                            git_sb[:, 128 * h:128 * (h + 1)],
                            id3_sb,
                        )
                # flush chunk to SBUF: src [128,(t:32)(g:3)] -> dst strided
                gi_view = gi_sb[:].rearrange("p (t g h) -> p t g h", g=3, h=NH)
                for h in range(NH):
                    nc.scalar.copy(
                        gi_view[:, tc_i * GI_CHUNK:(tc_i + 1) * GI_CHUNK, :, h],
                        gib[h][:].rearrange("p (t g) -> p t g", g=3),
                    )

            # ---- GRU scan ----
            w4 = cst_sb[:, 0:4]
            w2 = cst_sb[:, 4:5]
            b2 = cst_sb[:, 5:6]
            for t in range(T):
                h_prev = h0_sb if t == 0 else hout[:, 2 * (t - 1):2 * t]
                ghrz = scan.tile([128, 4], F32, tag="ghrz")
                nc.vector.tensor_tensor(ghrz, _bcast4(h_prev), w4, op=ALU.mult)
                grz = scan.tile([128, 4], F32, tag="grz")
                nc.vector.tensor_tensor(grz, ghrz, gi_sb[:, 6 * t:6 * t + 4], op=ALU.add)
                rz = scan.tile([128, 4], F32, tag="rz")
                nc.scalar.activation(out=rz, in_=grz, func=AF.Sigmoid)
                ghn = scan.tile([128, 2], F32, tag="ghn")
                nc.vector.tensor_scalar(
                    out=ghn, in0=h_prev, scalar1=w2, scalar2=b2,
                    op0=ALU.mult, op1=ALU.add,
                )
                nr = scan.tile([128, 2], F32, tag="nr")
                nc.vector.tensor_tensor(nr, rz[:, 0:2], ghn, op=ALU.mult)
                npre = scan.tile([128, 2], F32, tag="npre")
                nc.vector.tensor_tensor(npre, nr, gi_sb[:, 6 * t + 4:6 * t + 6], op=ALU.add)
                n_t = scan.tile([128, 2], F32, tag="n_t")
                nc.scalar.activation(out=n_t, in_=npre, func=AF.Tanh)
                d_t = scan.tile([128, 2], F32, tag="d_t")
                nc.vector.tensor_tensor(d_t, h_prev, n_t, op=ALU.subtract)
                zd = scan.tile([128, 2], F32, tag="zd")
                nc.vector.tensor_tensor(zd, rz[:, 2:4], d_t, op=ALU.mult)
                nc.vector.tensor_tensor(hout[:, 2 * t:2 * t + 2], n_t, zd, op=ALU.add)

            # ---- batch-sum over partitions ----
            sum_ps = sump.tile([1, T * NH], F32)
            nc.tensor.matmul(sum_ps, ones_sb[:], hout[:], start=True, stop=True)
            sum_sb = scan.tile([1, T * NH], F32)
            nc.vector.tensor_copy(sum_sb, sum_ps)
            nc.sync.dma_start(out=out[:], in_=sum_sb)

    _strip_same_engine_waits(nc)
    return nc


_ENG_PFX = {
    mybir.EngineType.Activation: "Activation",
    mybir.EngineType.DVE: "DVE",
    mybir.EngineType.PE: "PE",
    mybir.EngineType.Pool: "Pool",
    mybir.EngineType.SP: "SP",
}


def _strip_same_engine_waits(nc):
    """The compute-engine instruction formats have a single sync-wait slot.

    Tile's semaphore assignment is not transitively minimal and often adds a
    wait on the instruction's own engine semaphore next to a cross-engine
    wait. Engines execute their own stream in order, so same-engine waits
    are vacuous -- drop them when an instruction carries more than one wait.
    """
    multi = []
    for inst in nc.inst_map.values():
        si = inst.sync_info
        if not si or not si.on_wait or len(si.on_wait) <= 1:
            continue
        pfx = _ENG_PFX.get(inst.engine)
        if pfx is not None:
            kept = [
                w
                for w in si.on_wait
                if not (w.ant_name or "").startswith(pfx + "_")
            ]
            if len(kept) != len(si.on_wait):
                si.on_wait = kept
        if len(si.on_wait) > 1 and type(inst).__name__ == "InstDMACopy":
            # x-tile slot reuse: the WAW wait on the previous DMA's queue sem
            # is transitively covered by the WAR wait on the compute engine
            # that consumed the whole previous write.
            comp = [
                w
                for w in si.on_wait
                if not (w.ant_name or "").startswith(("DMAHW", "DMASW"))
            ]
            if comp:
                si.on_wait = comp
        if len(si.on_wait) > 1:
            multi.append((inst.name, type(inst).__name__, str(inst.engine),
                          [w.ant_name for w in si.on_wait]))

    # The kernel-tail SP drain waits on every engine + DMA queue at once;
    # the CTRL struct has a single wait slot, so split it into a chain of
    # single-wait drains.
    for block in nc.m.functions[0].blocks:
        insts = block.instructions
        for idx in range(len(insts) - 1, -1, -1):
            inst = insts[idx]
            si = inst.sync_info
            if (
                type(inst).__name__ != "InstDrain"
                or not si
                or not si.on_wait
                or len(si.on_wait) <= 1
            ):
                continue
            waits = list(si.on_wait)
            si.on_wait = waits[-1:]
            pre = []
            for k, w in enumerate(waits[:-1]):
                d = mybir.InstDrain(
                    name=f"{inst.name}-w{k}", ins=[], outs=[]
                )
                d.engine = inst.engine
                d.sync_info = mybir.SyncInfo(on_wait=[w], on_update=[])
                pre.append(d)
            insts[idx:idx] = pre
            multi = [m for m in multi if m[0] != inst.name]

    if multi:
        import sys
        print(f"[kernel] WARNING: {len(multi)} instructions still have >1 "
              f"sync wait: {multi[:8]}", file=sys.stderr)


def kernel(x, h0, w_ih, w_hh, b_ih, b_hh):
    x = np.asarray(x, dtype=np.float32)
    h0 = np.asarray(h0, dtype=np.float32)
    w_ih = np.asarray(w_ih, dtype=np.float32)
    w_hh = np.asarray(w_hh, dtype=np.float32)
    b_ih = np.asarray(b_ih, dtype=np.float32)
    b_hh = np.asarray(b_hh, dtype=np.float32)

    if "nc" not in _CACHE:
        _CACHE["nc"] = build_nc()
    nc = _CACHE["nc"]

    wT = np.ascontiguousarray(w_ih.T)                     # [D, 3]
    gb = np.array(
        [b_ih[0] + b_hh[0], b_ih[1] + b_hh[1], b_ih[2]], dtype=np.float32
    )
    w0, w1, w2 = float(w_hh[0, 0]), float(w_hh[1, 0]), float(w_hh[2, 0])
    wc = np.array([w0, w0, w1, w1, w2, float(b_hh[2])], dtype=np.float32)
    g3 = np.concatenate([gb[:, None], np.eye(3, dtype=np.float32)], axis=1)
    g3 = np.ascontiguousarray(g3, dtype=np.float32)       # [3, 4]

    in_maps = []
    for c in range(N_CORES):
        xs = x[c * B_LOC:(c + 1) * B_LOC]                 # [B_loc, T, D]
        xt = np.ascontiguousarray(xs.transpose(1, 2, 0))  # [T, D, B_loc]
        h0c = h0[0, c * B_LOC:(c + 1) * B_LOC, 0]         # [B_loc]
        h0t = h0c.reshape(NH, 128).T                      # [128, NH]
        cstc = np.ascontiguousarray(
            np.concatenate([np.broadcast_to(wc, (128, 6)), h0t], axis=1),
            dtype=np.float32,
        )
        in_maps.append({"xt": xt, "wT": wT, "cst": cstc, "g3": g3})

    res = run_bass_kernel_spmd(nc, in_maps, core_ids=list(range(N_CORES)))
    total = np.zeros((T * NH,), dtype=np.float64)
    for r in res.results:
        total += r["out"].reshape(-1).astype(np.float64)
    out = total.reshape(T, NH).sum(axis=1) / B
    return out.astype(np.float32)


# revision 14
# speedup vs baseline: 1.1693x; 1.1693x over previous
"""GRU (hidden_size=1) kernel for Trainium2, data-parallel over batch on 8 cores.

Problem: x[2048, 128, 512] f32, gi = x @ w_ih.T + b_ih, then a sequential
GRU scan over T=128 with scalar hidden state per batch element, output is
mean over batch of h_t -> [128].

Strategy per core (B_loc = 256 batch elements):
  - host pre-arranges the core's x slice as xt[T, D, B_loc] so DMA delivers
    [d, b] tiles directly (contraction dim on partitions, no on-chip
    transpose of x).
  - w-stationary float32r matmuls (full-rate at N>=256) over two timesteps
    at once produce giT[3, 2*b] in PSUM, accumulated over 4 d-chunks.
  - tiny PE transposes flip giT[3, 128] -> [128, 3] per batch half, giving
    gi with batch on partitions.
  - fp32 GRU scan over t with batch on partitions ([128, 2] element ops),
    emitted interleaved with production so the engines' static instruction
    order lets the scan trail production by one chunk.
  - partition-sum via ones-matmul gives per-(t, half) batch sums; host sums
    the 8 per-core partials and divides by B.
"""

import numpy as np

import concourse.bass as bass
import concourse.mybir as mybir
from concourse.bass_types import AP
from concourse.tile import TileContext
from concourse.bass_utils import run_bass_kernel_spmd

F32 = mybir.dt.float32
F32R = mybir.dt.float32r
AF = mybir.ActivationFunctionType
ALU = mybir.AluOpType

N_CORES = 8
B, T, D = 2048, 128, 512
B_LOC = B // N_CORES          # 256
NH = B_LOC // 128             # 2 batch halves per core
NCH = D // 128                # 4 contraction chunks
TP = 2                        # timesteps per matmul group (N = TP*B_LOC/... )
GI_CHUNK = 8                  # t-steps per PSUM->SBUF gi flush / interleave

_CACHE = {}


def _bcast4(h_sl):
    """[128, 2] AP -> [128, 2(bcast), 2] AP reading (h0,h1,h0,h1)."""
    return AP(
        tensor=h_sl.tensor,
        offset=h_sl.offset,
        ap=[h_sl.ap[0], [0, 2], h_sl.ap[1]],
    )


def build_nc():
    nc = bass.Bass(trn_type="TRN2")

    xt = nc.dram_tensor("xt", [T, D, B_LOC], F32, kind="ExternalInput")
    wT = nc.dram_tensor("wT", [D, 3], F32, kind="ExternalInput")
    cst = nc.dram_tensor("cst", [128, 6 + NH], F32, kind="ExternalInput")
    g3 = nc.dram_tensor("g3", [3, 4], F32, kind="ExternalInput")
    out = nc.dram_tensor("out", [1, T * NH], F32, kind="ExternalOutput")

    with TileContext(nc) as tc:
        with (
            tc.tile_pool(name="xpool", bufs=6) as xpool,
            tc.tile_pool(name="consts", bufs=1) as consts,
            tc.tile_pool(name="gits", bufs=4) as gits,
            tc.tile_pool(name="scan", bufs=1) as scan,
            tc.tile_pool(name="gtp", bufs=2, space="PSUM") as gtp,
            tc.tile_pool(name="gbp", bufs=2, space="PSUM") as gbp,
            tc.tile_pool(name="sump", bufs=1, space="PSUM") as sump,
        ):
            # ---- constants ----
            wT_sb = consts.tile([128, NCH, 3], F32R)
            nc.sync.dma_start(
                out=wT_sb,
                in_=wT[:].rearrange("(c p) g -> p c g", p=128).bitcast(F32R),
            )
            cst_sb = consts.tile([128, 6 + NH], F32)
            nc.sync.dma_start(out=cst_sb, in_=cst[:])
            g3_sb = consts.tile([3, 4], F32)
            nc.sync.dma_start(out=g3_sb, in_=g3[:])
            gb_sb = g3_sb[:, 0:1]
            id3_sb = g3_sb[:, 1:4]
            h0_sb = cst_sb[:, 6:6 + NH]
            ones_sb = consts.tile([128, 1], F32)
            nc.vector.memset(ones_sb, 1.0)

            # warm-up consumers: absorb each const-DMA semaphore into the
            # engines' vector clocks so later instructions need only one wait
            # (most instruction formats have a single sync-wait slot).
            warm_ps = sump.tile([3, 3], F32, tag="warm")
            nc.tensor.ldweights(wT_sb[:, 0, :].bitcast(mybir.dt.bfloat16))
            nc.tensor.transpose(warm_ps, id3_sb, id3_sb)
            warm_sb = consts.tile([3, 1], F32)
            nc.scalar.copy(warm_sb, g3_sb[:, 0:1])

            # ---- persistent buffers ----
            gi_sb = scan.tile([128, T * 3 * NH], F32)   # col 6t+2g+half
            hout = scan.tile([128, T * NH], F32)        # col 2t+half
            gi_view = gi_sb[:].rearrange("p (t g h) -> p t g h", g=3, h=NH)
            w4 = cst_sb[:, 0:4]
            w2 = cst_sb[:, 4:5]
            b2 = cst_sb[:, 5:6]

            def produce_chunk(tc_i):
                gib = [
                    gbp.tile(
                        [128, GI_CHUNK * 3], F32, tag=f"gib{h}", name=f"gib{h}"
                    )
                    for h in range(NH)
                ]
                for tp in range(GI_CHUNK // TP):
                    t0 = tc_i * GI_CHUNK + tp * TP
                    x_sb = xpool.tile([128, TP, NCH, B_LOC], F32R)
                    src = AP(
                        tensor=xt,
                        offset=t0 * D * B_LOC,
                        ap=[
                            [B_LOC, 128],            # p (d within chunk)
                            [D * B_LOC, TP],         # t within pair
                            [128 * B_LOC, NCH],      # d chunk
                            [1, B_LOC],              # b
                        ],
                    ).bitcast(F32R)
                    nc.sync.dma_start(out=x_sb, in_=src)
                    git_ps = gtp.tile([3, TP * B_LOC], F32)
                    for c in range(NCH):
                        nc.tensor.matmul(
                            git_ps,
                            wT_sb[:, c, :],
                            x_sb[:, :, c, :],
                            start=(c == 0),
                            stop=(c == NCH - 1),
                        )
                    git_sb = gits.tile([3, TP * B_LOC], F32)
                    nc.scalar.activation(
                        out=git_sb, in_=git_ps, func=AF.Identity,
                        bias=gb_sb, scale=1.0,
                    )
                    for ts in range(TP):
                        tt = tp * TP + ts
                        for h in range(NH):
                            nc.tensor.transpose(
                                gib[h][:, 3 * tt:3 * tt + 3],
                                git_sb[:, 256 * ts + 128 * h:256 * ts + 128 * (h + 1)],
                                id3_sb,
                            )
                # flush chunk to SBUF (strided dst: col 6t+2g+half)
                for h in range(NH):
                    nc.scalar.copy(
                        gi_view[:, tc_i * GI_CHUNK:(tc_i + 1) * GI_CHUNK, :, h],
                        gib[h][:].rearrange("p (t g) -> p t g", g=3),
                    )

            def scan_step(t):
                h_prev = h0_sb if t == 0 else hout[:, 2 * (t - 1):2 * t]
                ghrz = scan.tile([128, 4], F32, tag="ghrz", name="ghrz")
                nc.vector.tensor_tensor(ghrz, _bcast4(h_prev), w4, op=ALU.mult)
                grz = scan.tile([128, 4], F32, tag="grz", name="grz")
                nc.vector.tensor_tensor(
                    grz, ghrz, gi_sb[:, 6 * t:6 * t + 4], op=ALU.add
                )
                rz = scan.tile([128, 4], F32, tag="rz", name="rz")
                nc.scalar.activation(out=rz, in_=grz, func=AF.Sigmoid)
                ghn = scan.tile([128, 2], F32, tag="ghn", name="ghn")
                nc.vector.tensor_scalar(
                    out=ghn, in0=h_prev, scalar1=w2, scalar2=b2,
                    op0=ALU.mult, op1=ALU.add,
                )
                nr = scan.tile([128, 2], F32, tag="nr", name="nr")
                nc.vector.tensor_tensor(nr, rz[:, 0:2], ghn, op=ALU.mult)
                npre = scan.tile([128, 2], F32, tag="npre", name="npre")
                nc.vector.tensor_tensor(
                    npre, nr, gi_sb[:, 6 * t + 4:6 * t + 6], op=ALU.add
                )
                n_t = scan.tile([128, 2], F32, tag="n_t", name="n_t")
                nc.scalar.activation(out=n_t, in_=npre, func=AF.Tanh)
                d_t = scan.tile([128, 2], F32, tag="d_t", name="d_t")
                nc.vector.tensor_tensor(d_t, h_prev, n_t, op=ALU.subtract)
                zd = scan.tile([128, 2], F32, tag="zd", name="zd")
                nc.vector.tensor_tensor(zd, rz[:, 2:4], d_t, op=ALU.mult)
                nc.vector.tensor_tensor(
                    hout[:, 2 * t:2 * t + 2], n_t, zd, op=ALU.add
                )

            # ---- interleaved production + scan ----
            n_chunks = T // GI_CHUNK
            for tc_i in range(n_chunks):
                produce_chunk(tc_i)
                if tc_i >= 1:
                    for tt in range(GI_CHUNK):
                        scan_step((tc_i - 1) * GI_CHUNK + tt)
            for tt in range(GI_CHUNK):
                scan_step((n_chunks - 1) * GI_CHUNK + tt)

            # ---- batch-sum over partitions ----
            sum_ps = sump.tile([1, T * NH], F32)
            nc.tensor.matmul(sum_ps, ones_sb[:], hout[:], start=True, stop=True)
            sum_sb = scan.tile([1, T * NH], F32)
            nc.vector.tensor_copy(sum_sb, sum_ps)
            nc.sync.dma_start(out=out[:], in_=sum_sb)

    _strip_same_engine_waits(nc)
    return nc


_ENG_PFX = {
    mybir.EngineType.Activation: "Activation",
    mybir.EngineType.DVE: "DVE",
    mybir.EngineType.PE: "PE",
    mybir.EngineType.Pool: "Pool",
    mybir.EngineType.SP: "SP",
}


def _strip_same_engine_waits(nc):
    """The compute-engine instruction formats have a single sync-wait slot.

    Tile's semaphore assignment is not transitively minimal and often adds a
    wait on the instruction's own engine semaphore next to a cross-engine
    wait. Engines execute their own stream in order, so same-engine waits
    are vacuous -- drop them when an instruction carries more than one wait.
    """
    multi = []
    for inst in nc.inst_map.values():
        si = inst.sync_info
        if not si or not si.on_wait or len(si.on_wait) <= 1:
            continue
        pfx = _ENG_PFX.get(inst.engine)
        if pfx is not None:
            kept = [
                w
                for w in si.on_wait
                if not (w.ant_name or "").startswith(pfx + "_")
            ]
            if len(kept) != len(si.on_wait):
                si.on_wait = kept
        if len(si.on_wait) > 1 and type(inst).__name__ == "InstDMACopy":
            # x-tile slot reuse: the WAW wait on the previous DMA's queue sem
            # is transitively covered by the WAR wait on the compute engine
            # that consumed the whole previous write.
            comp = [
                w
                for w in si.on_wait
                if not (w.ant_name or "").startswith(("DMAHW", "DMASW"))
            ]
            if comp:
                si.on_wait = comp
        if len(si.on_wait) > 1:
            multi.append((inst.name, type(inst).__name__, str(inst.engine),
                          [w.ant_name for w in si.on_wait]))

    # The kernel-tail SP drain waits on every engine + DMA queue at once;
    # the CTRL struct has a single wait slot, so split it into a chain of
    # single-wait drains.
    for block in nc.m.functions[0].blocks:
        insts = block.instructions
        for idx in range(len(insts) - 1, -1, -1):
            inst = insts[idx]
            si = inst.sync_info
            if (
                type(inst).__name__ != "InstDrain"
                or not si
                or not si.on_wait
                or len(si.on_wait) <= 1
            ):
                continue
            waits = list(si.on_wait)
            si.on_wait = waits[-1:]
            pre = []
            for k, w in enumerate(waits[:-1]):
                d = mybir.InstDrain(
                    name=f"{inst.name}-w{k}", ins=[], outs=[]
                )
                d.engine = inst.engine
                d.sync_info = mybir.SyncInfo(on_wait=[w], on_update=[])
                pre.append(d)
            insts[idx:idx] = pre
            multi = [m for m in multi if m[0] != inst.name]

    if multi:
        import sys
        print(f"[kernel] WARNING: {len(multi)} instructions still have >1 "
              f"sync wait: {multi[:8]}", file=sys.stderr)


def kernel(x, h0, w_ih, w_hh, b_ih, b_hh):
    x = np.asarray(x, dtype=np.float32)
    h0 = np.asarray(h0, dtype=np.float32)
    w_ih = np.asarray(w_ih, dtype=np.float32)
    w_hh = np.asarray(w_hh, dtype=np.float32)
    b_ih = np.asarray(b_ih, dtype=np.float32)
    b_hh = np.asarray(b_hh, dtype=np.float32)

    if "nc" not in _CACHE:
        _CACHE["nc"] = build_nc()
    nc = _CACHE["nc"]

    wT = np.ascontiguousarray(w_ih.T)                     # [D, 3]
    gb = np.array(
        [b_ih[0] + b_hh[0], b_ih[1] + b_hh[1], b_ih[2]], dtype=np.float32
    )
    w0, w1, w2 = float(w_hh[0, 0]), float(w_hh[1, 0]), float(w_hh[2, 0])
    wc = np.array([w0, w0, w1, w1, w2, float(b_hh[2])], dtype=np.float32)
    g3 = np.concatenate([gb[:, None], np.eye(3, dtype=np.float32)], axis=1)
    g3 = np.ascontiguousarray(g3, dtype=np.float32)       # [3, 4]

    in_maps = []
    for c in range(N_CORES):
        xs = x[c * B_LOC:(c + 1) * B_LOC]                 # [B_loc, T, D]
        xt = np.ascontiguousarray(xs.transpose(1, 2, 0))  # [T, D, B_loc]
        h0c = h0[0, c * B_LOC:(c + 1) * B_LOC, 0]         # [B_loc]
        h0t = h0c.reshape(NH, 128).T                      # [128, NH]
        cstc = np.ascontiguousarray(
            np.concatenate([np.broadcast_to(wc, (128, 6)), h0t], axis=1),
            dtype=np.float32,
        )
        in_maps.append({"xt": xt, "wT": wT, "cst": cstc, "g3": g3})

    res = run_bass_kernel_spmd(nc, in_maps, core_ids=list(range(N_CORES)))
    total = np.zeros((T * NH,), dtype=np.float64)
    for r in res.results:
        total += r["out"].reshape(-1).astype(np.float64)
    out = total.reshape(T, NH).sum(axis=1) / B
    return out.astype(np.float32)


# revision 17
# speedup vs baseline: 1.2579x; 1.0758x over previous
"""GRU (hidden_size=1) kernel for Trainium2, data-parallel over batch on 8 cores.

Problem: x[2048, 128, 512] f32, gi = x @ w_ih.T + b_ih, then a sequential
GRU scan over T=128 with scalar hidden state per batch element, output is
mean over batch of h_t -> [128].

Strategy per core (B_loc = 256 batch elements):
  - host pre-arranges the core's x slice as xt[T, D, B_loc] so DMA delivers
    [d, b] tiles directly (contraction dim on partitions, no on-chip
    transpose of x).
  - w-stationary float32r matmuls (full-rate at N>=256) over two timesteps
    at once produce giT[3, 2*b] in PSUM, accumulated over 4 d-chunks.
  - tiny PE transposes flip giT[3, 128] -> [128, 3] per batch half, giving
    gi with batch on partitions.
  - fp32 GRU scan over t with batch on partitions ([128, 2] element ops),
    emitted interleaved with production so the engines' static instruction
    order lets the scan trail production by one chunk.
  - partition-sum via ones-matmul gives per-(t, half) batch sums; host sums
    the 8 per-core partials and divides by B.
"""

import numpy as np

import concourse.bass as bass
import concourse.mybir as mybir
from concourse.bass_types import AP
from concourse.tile import TileContext
from concourse.bass_utils import run_bass_kernel_spmd

F32 = mybir.dt.float32
F32R = mybir.dt.float32r
AF = mybir.ActivationFunctionType
ALU = mybir.AluOpType

N_CORES = 8
B, T, D = 2048, 128, 512
B_LOC = B // N_CORES          # 256
NH = B_LOC // 128             # 2 batch halves per core
NCH = D // 128                # 4 contraction chunks
TP = 2                        # timesteps per matmul group (N = TP*B_LOC/... )
GI_CHUNK = 8                  # t-steps per PSUM->SBUF gi flush / interleave

_CACHE = {}


def _bcast4(h_sl):
    """[128, 2] AP -> [128, 2(bcast), 2] AP reading (h0,h1,h0,h1)."""
    return AP(
        tensor=h_sl.tensor,
        offset=h_sl.offset,
        ap=[h_sl.ap[0], [0, 2], h_sl.ap[1]],
    )


def build_nc():
    nc = bass.Bass(trn_type="TRN2")

    xt = nc.dram_tensor("xt", [T, D, B_LOC], F32, kind="ExternalInput")
    wT = nc.dram_tensor("wT", [D, 3], F32, kind="ExternalInput")
    cst = nc.dram_tensor("cst", [128, 7 + NH], F32, kind="ExternalInput")
    g3 = nc.dram_tensor("g3", [3, 4], F32, kind="ExternalInput")
    out = nc.dram_tensor("out", [1, T * NH], F32, kind="ExternalOutput")

    with TileContext(nc) as tc:
        with (
            tc.tile_pool(name="xpool", bufs=6) as xpool,
            tc.tile_pool(name="consts", bufs=1) as consts,
            tc.tile_pool(name="gits", bufs=4) as gits,
            tc.tile_pool(name="scan", bufs=1) as scan,
            tc.tile_pool(name="gtp", bufs=2, space="PSUM") as gtp,
            tc.tile_pool(name="gbp", bufs=2, space="PSUM") as gbp,
            tc.tile_pool(name="sump", bufs=1, space="PSUM") as sump,
        ):
            # ---- constants ----
            wT_sb = consts.tile([128, NCH, 3], F32R)
            nc.sync.dma_start(
                out=wT_sb,
                in_=wT[:].rearrange("(c p) g -> p c g", p=128).bitcast(F32R),
            )
            cst_sb = consts.tile([128, 7 + NH], F32)
            nc.sync.dma_start(out=cst_sb, in_=cst[:])
            g3_sb = consts.tile([3, 4], F32)
            nc.sync.dma_start(out=g3_sb, in_=g3[:])
            gb_sb = g3_sb[:, 0:1]
            id3_sb = g3_sb[:, 1:4]
            h0_sb = cst_sb[:, 6:6 + NH]
            b2c = cst_sb[:, 6 + NH:7 + NH]
            ones_sb = consts.tile([128, 1], F32)
            nc.vector.memset(ones_sb, 1.0)
            ones2 = consts.tile([128, 2], F32)
            nc.vector.memset(ones2, 1.0)

            # warm-up consumers: absorb each const-DMA semaphore into the
            # engines' vector clocks so later instructions need only one wait
            # (most instruction formats have a single sync-wait slot).
            warm_ps = sump.tile([3, 3], F32, tag="warm")
            nc.tensor.ldweights(wT_sb[:, 0, :].bitcast(mybir.dt.bfloat16))
            nc.tensor.transpose(warm_ps, id3_sb, id3_sb)
            warm_sb = consts.tile([3, 1], F32)
            nc.scalar.copy(warm_sb, g3_sb[:, 0:1])

            # ---- persistent buffers ----
            gi_sb = scan.tile([128, T * 4 * NH], F32)   # 8/t: r0 r1 z0 z1 b2 b2 n0 n1
            hout = scan.tile([128, T * NH], F32)        # col 2t+half
            gi_view = gi_sb[:].rearrange("p (t g h) -> p t g h", g=4, h=NH)
            w6 = cst_sb[:, 0:6]
            # fill the constant b_hh[2] columns (cols 8t+4, 8t+5 for all t)
            nc.scalar.copy(
                gi_view[:, :, 2, :],
                AP(tensor=b2c.tensor, offset=b2c.offset,
                   ap=[b2c.ap[0], [0, T], [0, NH]]),
            )

            def produce_pair(gib, tc_i, tp):
                t0 = tc_i * GI_CHUNK + tp * TP
                x_sb = xpool.tile([128, TP, NCH, B_LOC], F32R, name="x_sb")
                src = AP(
                    tensor=xt,
                    offset=t0 * D * B_LOC,
                    ap=[
                        [B_LOC, 128],            # p (d within chunk)
                        [D * B_LOC, TP],         # t within pair
                        [128 * B_LOC, NCH],      # d chunk
                        [1, B_LOC],              # b
                    ],
                ).bitcast(F32R)
                nc.sync.dma_start(out=x_sb, in_=src)
                git_ps = gtp.tile([3, TP * B_LOC], F32, name="git_ps")
                for c in range(NCH):
                    nc.tensor.matmul(
                        git_ps,
                        wT_sb[:, c, :],
                        x_sb[:, :, c, :],
                        start=(c == 0),
                        stop=(c == NCH - 1),
                    )
                git_sb = gits.tile([3, TP * B_LOC], F32, name="git_sb")
                nc.scalar.activation(
                    out=git_sb, in_=git_ps, func=AF.Identity,
                    bias=gb_sb, scale=1.0,
                )
                for ts in range(TP):
                    tt = tp * TP + ts
                    for h in range(NH):
                        nc.tensor.transpose(
                            gib[h][:, 3 * tt:3 * tt + 3],
                            git_sb[:, 256 * ts + 128 * h:256 * ts + 128 * (h + 1)],
                            id3_sb,
                        )

            def flush_chunk(gib, tc_i):
                # flush chunk to SBUF: (r,z) -> cols 8t+{0..3}, n -> 8t+{6,7}
                csl = slice(tc_i * GI_CHUNK, (tc_i + 1) * GI_CHUNK)
                for h in range(NH):
                    gsrc = gib[h][:].rearrange("p (t g) -> p t g", g=3)
                    nc.scalar.copy(gi_view[:, csl, 0:2, h], gsrc[:, :, 0:2])
                    nc.scalar.copy(gi_view[:, csl, 3, h], gsrc[:, :, 2])

            def scan_step(t):
                h_prev = h0_sb if t == 0 else hout[:, 2 * (t - 1):2 * t]
                h6 = AP(tensor=h_prev.tensor, offset=h_prev.offset,
                        ap=[h_prev.ap[0], [0, 3], h_prev.ap[1]])
                gh6 = scan.tile([128, 6], F32, tag="gh6", name="gh6")
                nc.vector.tensor_tensor(gh6, h6, w6, op=ALU.mult)
                acc6 = scan.tile([128, 6], F32, tag="acc6", name="acc6")
                nc.vector.tensor_tensor(
                    acc6, gh6, gi_sb[:, 8 * t:8 * t + 6], op=ALU.add
                )
                rz = scan.tile([128, 4], F32, tag="rz", name="rz")
                nc.scalar.activation(out=rz, in_=acc6[:, 0:4], func=AF.Sigmoid)
                nr = scan.tile([128, 2], F32, tag="nr", name="nr")
                nc.vector.tensor_tensor(nr, rz[:, 0:2], acc6[:, 4:6], op=ALU.mult)
                npre = scan.tile([128, 2], F32, tag="npre", name="npre")
                nc.vector.tensor_tensor(
                    npre, nr, gi_sb[:, 8 * t + 6:8 * t + 8], op=ALU.add
                )
                n_t = scan.tile([128, 2], F32, tag="n_t", name="n_t")
                nc.scalar.activation(out=n_t, in_=npre, func=AF.Tanh)
                # fill the tanh window on DVE with the independent z-products
                zh = scan.tile([128, 2], F32, tag="zh", name="zh")
                nc.vector.tensor_tensor(zh, rz[:, 2:4], h_prev, op=ALU.mult)
                u_t = scan.tile([128, 2], F32, tag="u_t", name="u_t")
                nc.vector.tensor_tensor(u_t, ones2, rz[:, 2:4], op=ALU.subtract)
                nu = scan.tile([128, 2], F32, tag="nu", name="nu")
                nc.vector.tensor_tensor(nu, n_t, u_t, op=ALU.mult)
                nc.vector.tensor_tensor(
                    hout[:, 2 * t:2 * t + 2], nu, zh, op=ALU.add
                )

            # ---- interleaved production + scan ----
            # Scan steps of chunk c-1 are emitted BEFORE each production pair
            # of chunk c so ACT's static order runs [sigmoid, tanh, ...] ahead
            # of the giT copy; the copy then fills the DVE-phase window.
            n_chunks = T // GI_CHUNK
            n_pairs = GI_CHUNK // TP
            for tc_i in range(n_chunks):
                gib = [
                    gbp.tile(
                        [128, GI_CHUNK * 3], F32, tag=f"gib{h}", name=f"gib{h}"
                    )
                    for h in range(NH)
                ]
                for tp in range(n_pairs):
                    if tc_i >= 1:
                        for ts in range(TP):
                            scan_step((tc_i - 1) * GI_CHUNK + tp * TP + ts)
                    produce_pair(gib, tc_i, tp)
                flush_chunk(gib, tc_i)
            for tt in range(GI_CHUNK):
                scan_step((n_chunks - 1) * GI_CHUNK + tt)

            # ---- batch-sum over partitions ----
            sum_ps = sump.tile([1, T * NH], F32)
            nc.tensor.matmul(sum_ps, ones_sb[:], hout[:], start=True, stop=True)
            sum_sb = scan.tile([1, T * NH], F32)
            nc.vector.tensor_copy(sum_sb, sum_ps)
            nc.sync.dma_start(out=out[:], in_=sum_sb)

    _strip_same_engine_waits(nc)
    return nc


_ENG_PFX = {
    mybir.EngineType.Activation: "Activation",
    mybir.EngineType.DVE: "DVE",
    mybir.EngineType.PE: "PE",
    mybir.EngineType.Pool: "Pool",
    mybir.EngineType.SP: "SP",
}


def _strip_same_engine_waits(nc):
    """The compute-engine instruction formats have a single sync-wait slot.

    Tile's semaphore assignment is not transitively minimal and often adds a
    wait on the instruction's own engine semaphore next to a cross-engine
    wait. Engines execute their own stream in order, so same-engine waits
    are vacuous -- drop them when an instruction carries more than one wait.
    """
    multi = []
    for inst in nc.inst_map.values():
        si = inst.sync_info
        if not si or not si.on_wait or len(si.on_wait) <= 1:
            continue
        pfx = _ENG_PFX.get(inst.engine)
        if pfx is not None:
            kept = [
                w
                for w in si.on_wait
                if not (w.ant_name or "").startswith(pfx + "_")
            ]
            if len(kept) != len(si.on_wait):
                si.on_wait = kept
        if len(si.on_wait) > 1 and type(inst).__name__ == "InstDMACopy":
            # x-tile slot reuse: the WAW wait on the previous DMA's queue sem
            # is transitively covered by the WAR wait on the compute engine
            # that consumed the whole previous write.
            comp = [
                w
                for w in si.on_wait
                if not (w.ant_name or "").startswith(("DMAHW", "DMASW"))
            ]
            if comp:
                si.on_wait = comp
        if len(si.on_wait) > 1:
            multi.append((inst.name, type(inst).__name__, str(inst.engine),
                          [w.ant_name for w in si.on_wait]))

    # The kernel-tail SP drain waits on every engine + DMA queue at once;
    # the CTRL struct has a single wait slot, so split it into a chain of
    # single-wait drains.
    for block in nc.m.functions[0].blocks:
        insts = block.instructions
        for idx in range(len(insts) - 1, -1, -1):
            inst = insts[idx]
            si = inst.sync_info
            if (
                type(inst).__name__ != "InstDrain"
                or not si
                or not si.on_wait
                or len(si.on_wait) <= 1
            ):
                continue
            waits = list(si.on_wait)
            si.on_wait = waits[-1:]
            pre = []
            for k, w in enumerate(waits[:-1]):
                d = mybir.InstDrain(
                    name=f"{inst.name}-w{k}", ins=[], outs=[]
                )
                d.engine = inst.engine
                d.sync_info = mybir.SyncInfo(on_wait=[w], on_update=[])
                pre.append(d)
            insts[idx:idx] = pre
            multi = [m for m in multi if m[0] != inst.name]

    if multi:
        import sys
        print(f"[kernel] WARNING: {len(multi)} instructions still have >1 "
              f"sync wait: {multi[:8]}", file=sys.stderr)


def kernel(x, h0, w_ih, w_hh, b_ih, b_hh):
    x = np.asarray(x, dtype=np.float32)
    h0 = np.asarray(h0, dtype=np.float32)
    w_ih = np.asarray(w_ih, dtype=np.float32)
    w_hh = np.asarray(w_hh, dtype=np.float32)
    b_ih = np.asarray(b_ih, dtype=np.float32)
    b_hh = np.asarray(b_hh, dtype=np.float32)

    if "nc" not in _CACHE:
        _CACHE["nc"] = build_nc()
    nc = _CACHE["nc"]

    wT = np.ascontiguousarray(w_ih.T)                     # [D, 3]
    gb = np.array(
        [b_ih[0] + b_hh[0], b_ih[1] + b_hh[1], b_ih[2]], dtype=np.float32
    )
    w0, w1, w2 = float(w_hh[0, 0]), float(w_hh[1, 0]), float(w_hh[2, 0])
    wc = np.array([w0, w0, w1, w1, w2, w2], dtype=np.float32)
    b2v = np.full((128, 1), float(b_hh[2]), dtype=np.float32)
    g3 = np.concatenate([gb[:, None], np.eye(3, dtype=np.float32)], axis=1)
    g3 = np.ascontiguousarray(g3, dtype=np.float32)       # [3, 4]

    in_maps = []
    for c in range(N_CORES):
        xs = x[c * B_LOC:(c + 1) * B_LOC]                 # [B_loc, T, D]
        xt = np.ascontiguousarray(xs.transpose(1, 2, 0))  # [T, D, B_loc]
        h0c = h0[0, c * B_LOC:(c + 1) * B_LOC, 0]         # [B_loc]
        h0t = h0c.reshape(NH, 128).T                      # [128, NH]
        cstc = np.ascontiguousarray(
            np.concatenate([np.broadcast_to(wc, (128, 6)), h0t, b2v], axis=1),
            dtype=np.float32,
        )
        in_maps.append({"xt": xt, "wT": wT, "cst": cstc, "g3": g3})

    res = run_bass_kernel_spmd(nc, in_maps, core_ids=list(range(N_CORES)))
    total = np.zeros((T * NH,), dtype=np.float64)
    for r in res.results:
        total += r["out"].reshape(-1).astype(np.float64)
    out = total.reshape(T, NH).sum(axis=1) / B
    return out.astype(np.float32)
